# revision 21
# baseline (speedup 1.0000x reference)
"""Trainium2 Bass kernel for DynamicViewSelection.

Contract: kernel(**full_inputs) -> (selected_features (4,512) f32,
                                    indices (4,) int32,
                                    rendered (1024,224,224,3) f32)

Observations driving the design:
  * `rendered` is input-independent: the reference renderer ignores the
    camera positions and the point cloud; every view is the same constant
    (224,224,3) radial-falloff image. The kernel's dominant cost is the
    memory-bound broadcast-write of 1024 copies (616 MB) to HBM.
  * Scoring (cosine vs text + 2-layer MLP + weighted utility) is tiny:
    (1024,512) inputs. Shard candidates 128-per-core, compute utility on
    device, gather the 1024-vector on host for the global top-4.

Sharding: candidates split contiguously across 8 cores (core c handles
views [128c, 128c+128)). MLP weights / text features / scalar weights are
replicated. Per-core output: 77 MB rendered slice + (128,) utility.
"""

import numpy as np
from contextlib import ExitStack

NCORES = 8
C = 1024                 # num_candidates
D = 512                  # feature dim
HID = 256                # MLP hidden
CP = C // NCORES         # 128 candidates per core
IMG = 224
VIEW = IMG * IMG * 3     # 150528 elements per rendered view
ROWS_PER_VIEW = 32       # partition rows one view occupies in the SBUF tile
EPS = 1e-8

_CACHE = {}


def _base_image():
    """The constant rendered view, exactly as the reference computes it.

    Uses jax (same ops as the reference renderer) so the constant matches
    the reference bit-for-bit under the same jax backend; falls back to
    numpy (~1e-5 absmax difference) if jax is unavailable.
    """
    try:
        import jax.numpy as jnp
        coords = (jnp.arange(IMG, dtype=jnp.float32) - 112.0) / 112.0
        dist = jnp.sqrt(coords[:, None] ** 2 + coords[None, :] ** 2)
        intensity = jnp.exp(-dist * 2.0)
        chan = jnp.asarray([1.0, 0.8, 0.6], dtype=jnp.float32)
        return np.asarray(intensity[:, :, None] * chan, dtype=np.float32)
    except Exception:
        coords = ((np.arange(IMG, dtype=np.float32) - np.float32(112.0))
                  / np.float32(112.0)).astype(np.float32)
        dist = np.sqrt(coords[:, None] ** 2
                       + coords[None, :] ** 2).astype(np.float32)
        intensity = np.exp(-dist * np.float32(2.0)).astype(np.float32)
        chan = np.array([1.0, 0.8, 0.6], dtype=np.float32)
        return (intensity[:, :, None] * chan).astype(np.float32)


def _build(fanout_repeat=1, rows_per_view=ROWS_PER_VIEW, engines="sa:2"):
    import concourse.bass as bass
    import concourse.tile as tile
    from concourse import bacc, mybir

    kk = 128 // rows_per_view       # views held in the SBUF image tile
    cols = VIEW // rows_per_view    # f32 per partition in the tile
    nblk = CP // kk                 # fanout passes

    F32 = mybir.dt.float32
    AF = mybir.ActivationFunctionType
    ALU = mybir.AluOpType
    ts = bass.ts

    nc = bacc.Bacc("TRN2", target_bir_lowering=False, debug=False,
                   num_devices=NCORES)

    vf = nc.dram_tensor("vf", [CP, D], F32, kind="ExternalInput").ap()
    vfT = nc.dram_tensor("vfT", [D, CP], F32, kind="ExternalInput").ap()
    txt = nc.dram_tensor("txt", [128, D], F32, kind="ExternalInput").ap()
    w1d = nc.dram_tensor("w1d", [D, HID], F32, kind="ExternalInput").ap()
    b1r = nc.dram_tensor("b1r", [128, HID], F32, kind="ExternalInput").ap()
    w2r = nc.dram_tensor("w2r", [128, HID], F32, kind="ExternalInput").ap()
    b2r = nc.dram_tensor("b2r", [128, 1], F32, kind="ExternalInput").ap()
    scl = nc.dram_tensor("scl", [128, 4], F32, kind="ExternalInput").ap()
    imgb = nc.dram_tensor("imgb", [128, cols], F32, kind="ExternalInput").ap()

    nsets = min(fanout_repeat, 3)  # >1 only for benchmarking
    rend = nc.dram_tensor("rend", [nsets * nblk, 128 * cols], F32,
                          kind="ExternalOutput").ap()
    util = nc.dram_tensor("util", [128, 1], F32, kind="ExternalOutput").ap()

    with tile.TileContext(nc) as tc:
        with ExitStack() as ctx:
            imgp = ctx.enter_context(tc.tile_pool(name="imgp", bufs=1))
            sp = ctx.enter_context(tc.tile_pool(name="sp", bufs=1))
            pp = ctx.enter_context(tc.tile_pool(name="pp", bufs=1, space="PSUM"))

            # --- scoring inputs on the SWDGE ring (off the bulk path) ---
            vf_t = sp.tile([128, D], F32)
            nc.gpsimd.dma_start(vf_t[:], vf[:])
            vfT_t = sp.tile([128, D], F32)
            w1_t = sp.tile([128, 4 * HID], F32)
            for k in range(4):
                nc.gpsimd.dma_start(vfT_t[:, ts(k, 128)], vfT[ts(k, 128), :])
                nc.gpsimd.dma_start(w1_t[:, ts(k, HID)], w1d[ts(k, 128), :])
            txt_t = sp.tile([128, D], F32)
            nc.gpsimd.dma_start(txt_t[:], txt[:])
            b1_t = sp.tile([128, HID], F32)
            nc.gpsimd.dma_start(b1_t[:], b1r[:])
            w2_t = sp.tile([128, HID], F32)
            nc.gpsimd.dma_start(w2_t[:], w2r[:])
            b2_t = sp.tile([128, 1], F32)
            nc.gpsimd.dma_start(b2_t[:], b2r[:])
            sc_t = sp.tile([128, 4], F32)
            nc.gpsimd.dma_start(sc_t[:], scl[:])

            # --- bulk: image block in once, fan out to all view slots ---
            eng_map = {"s": nc.sync, "a": nc.scalar, "g": nc.gpsimd}
            rot = [eng_map[ch] for ch in engines if ch in eng_map]
            csplit = int(engines.split(":")[1]) if ":" in engines else 1
            cw = cols // csplit
            img_t = imgp.tile([128, cols], F32)
            # split the load along the same column strips as the fanout, and
            # issue each strip's load on the ring that will write that strip
            # (rings are FIFO, so a same-ring load ahead of unrelated writes
            # would stall them) — strip s's writes start as soon as strip s
            # has landed
            for s in range(csplit):
                rot[s % len(rot)].dma_start(img_t[:, s * cw:(s + 1) * cw],
                                            imgb[:, s * cw:(s + 1) * cw])
            rend3 = rend.rearrange("b (p c) -> b p c", p=128)
            i = 0
            for r in range(fanout_repeat):  # >1 only for benchmarking
                off = (r % nsets) * nblk
                for b in range(nblk):
                    if csplit == 1:
                        rot[i % len(rot)].dma_start(rend[off + b], img_t[:])
                        i += 1
                    else:
                        for s in range(csplit):
                            rot[i % len(rot)].dma_start(
                                rend3[off + b, :, s * cw:(s + 1) * cw],
                                img_t[:, s * cw:(s + 1) * cw])
                            i += 1

            # --- cosine(view_features, text) ---
            # (tensor_tensor_reduce crashes the exec unit on this runtime
            #  path, so reductions are mul + tensor_reduce pairs)
            scr1 = sp.tile([128, D], F32)
            numv = sp.tile([128, 1], F32)
            nc.vector.tensor_mul(scr1[:], vf_t[:], txt_t[:])
            nc.vector.tensor_reduce(numv[:], scr1[:],
                                    axis=mybir.AxisListType.X, op=ALU.add)
            scr2 = sp.tile([128, D], F32)
            ssq = sp.tile([128, 1], F32)
            nc.vector.tensor_mul(scr2[:], vf_t[:], vf_t[:])
            nc.vector.tensor_reduce(ssq[:], scr2[:],
                                    axis=mybir.AxisListType.X, op=ALU.add)
            scr3 = sp.tile([128, D], F32)
            tsq = sp.tile([128, 1], F32)
            nc.vector.tensor_mul(scr3[:], txt_t[:], txt_t[:])
            nc.vector.tensor_reduce(tsq[:], scr3[:],
                                    axis=mybir.AxisListType.X, op=ALU.add)
            nvf = sp.tile([128, 1], F32)
            nc.scalar.sqrt(nvf[:], ssq[:])
            ntx = sp.tile([128, 1], F32)
            nc.scalar.sqrt(ntx[:], tsq[:])
            den = sp.tile([128, 1], F32)
            nc.vector.tensor_mul(den[:], nvf[:], ntx[:])
            denc = sp.tile([128, 1], F32)
            nc.vector.tensor_scalar_max(denc[:], den[:], EPS)
            rcp = sp.tile([128, 1], F32)
            nc.vector.reciprocal(rcp[:], denc[:])
            trel = sp.tile([128, 1], F32)
            nc.vector.tensor_mul(trel[:], numv[:], rcp[:])

            # --- coverage MLP: sigmoid(relu(vf @ w1 + b1) @ w2 + b2) ---
            hp = pp.tile([128, HID], F32)
            for k in range(4):
                nc.tensor.matmul(hp[:], vfT_t[:, ts(k, 128)], w1_t[:, ts(k, HID)],
                                 start=(k == 0), stop=(k == 3))
            hb = sp.tile([128, HID], F32)
            nc.vector.tensor_add(hb[:], hp[:], b1_t[:])
            h = sp.tile([128, HID], F32)
            nc.scalar.activation(h[:], hb[:], AF.Relu)
            scr4 = sp.tile([128, HID], F32)
            cpre = sp.tile([128, 1], F32)
            nc.vector.tensor_mul(scr4[:], h[:], w2_t[:])
            nc.vector.tensor_reduce(cpre[:], scr4[:],
                                    axis=mybir.AxisListType.X, op=ALU.add)
            cov = sp.tile([128, 1], F32)
            nc.scalar.activation(cov[:], cpre[:], AF.Sigmoid, bias=b2_t[:, 0:1])

            # --- utility = wt*trel + wc_n*cov + wclip_n*(0.8*trel) ---
            wsum = sp.tile([128, 1], F32)
            nc.vector.tensor_add(wsum[:], sc_t[:, 1:2], sc_t[:, 2:3])
            wrcp = sp.tile([128, 1], F32)
            nc.vector.reciprocal(wrcp[:], wsum[:])
            wcn = sp.tile([128, 1], F32)
            nc.vector.tensor_mul(wcn[:], sc_t[:, 1:2], wrcp[:])
            wcl = sp.tile([128, 1], F32)
            nc.vector.tensor_mul(wcl[:], sc_t[:, 2:3], wrcp[:])
            ca = sp.tile([128, 1], F32)
            nc.scalar.mul(ca[:], trel[:], 0.8)
            t0 = sp.tile([128, 1], F32)
            nc.vector.tensor_mul(t0[:], sc_t[:, 0:1], trel[:])
            t1 = sp.tile([128, 1], F32)
            nc.vector.tensor_mul(t1[:], wcn[:], cov[:])
            t2 = sp.tile([128, 1], F32)
            nc.vector.tensor_mul(t2[:], wcl[:], ca[:])
            u01 = sp.tile([128, 1], F32)
            nc.vector.tensor_add(u01[:], t0[:], t1[:])
            u = sp.tile([128, 1], F32)
            nc.vector.tensor_add(u[:], u01[:], t2[:])
            nc.gpsimd.dma_start(util[:], u[:])

    nc.compile()
    return nc


def _get_module():
    if "nc" not in _CACHE:
        _CACHE["nc"] = _build()
    return _CACHE["nc"]


def _make_in_maps(view_features, text_features, w1, b1, w2, b2, wt, wc, wclip,
                  rows_per_view=ROWS_PER_VIEW):
    vf_full = np.ascontiguousarray(np.asarray(view_features, dtype=np.float32))
    txt_full = np.asarray(text_features, dtype=np.float32).reshape(1, D)
    w1_np = np.ascontiguousarray(np.asarray(w1, dtype=np.float32))
    b1_np = np.asarray(b1, dtype=np.float32).reshape(HID)
    w2_np = np.asarray(w2, dtype=np.float32).reshape(HID)
    b2_np = np.asarray(b2, dtype=np.float32).reshape(1)

    img_flat = _base_image().reshape(-1)
    kk = 128 // rows_per_view
    imgb = np.ascontiguousarray(
        np.tile(img_flat.reshape(rows_per_view, VIEW // rows_per_view),
                (kk, 1)))

    txt_rep = np.ascontiguousarray(np.broadcast_to(txt_full, (128, D)))
    b1_rep = np.ascontiguousarray(np.broadcast_to(b1_np, (128, HID)))
    w2_rep = np.ascontiguousarray(np.broadcast_to(w2_np, (128, HID)))
    b2_rep = np.full((128, 1), b2_np[0], dtype=np.float32)
    sc = np.zeros((128, 4), dtype=np.float32)
    sc[:, 0] = np.float32(np.asarray(wt))
    sc[:, 1] = np.float32(np.asarray(wc))
    sc[:, 2] = np.float32(np.asarray(wclip))

    in_maps = []
    for c in range(NCORES):
        sl = vf_full[c * CP:(c + 1) * CP]
        in_maps.append({
            "vf": np.ascontiguousarray(sl),
            "vfT": np.ascontiguousarray(sl.T),
            "txt": txt_rep, "w1d": w1_np, "b1r": b1_rep, "w2r": w2_rep,
            "b2r": b2_rep, "scl": sc, "imgb": imgb,
        })
    return vf_full, in_maps


def _assemble(vf_full, outs):
    utility = np.concatenate([outs[c]["util"].reshape(CP)
                              for c in range(NCORES)])
    idx = np.argsort(-utility, kind="stable")[:4].astype(np.int32)
    selected = np.ascontiguousarray(vf_full[idx])
    rendered = np.empty((C, IMG, IMG, 3), dtype=np.float32)
    for c in range(NCORES):
        rendered[c * CP:(c + 1) * CP] = outs[c]["rend"].reshape(CP, IMG, IMG, 3)
    return selected, idx, rendered


def run_raw(in_maps, **kwargs):
    """Run the compiled SPMD kernel; kwargs forwarded (e.g. trace=True)."""
    from concourse.bass_utils import run_bass_kernel_spmd
    nc = _get_module()
    return run_bass_kernel_spmd(nc, in_maps, core_ids=list(range(NCORES)),
                                **kwargs)


def kernel(point_cloud, view_features, text_features, w1, b1, w2, b2,
           wt, wc, wclip, num_candidates=1024, num_views=4):
    assert int(num_candidates) == C and int(num_views) == 4
    vf_full, in_maps = _make_in_maps(view_features, text_features,
                                     w1, b1, w2, b2, wt, wc, wclip)
    res = run_raw(in_maps)
    return _assemble(vf_full, res.results)
